# revision 1
# baseline (speedup 1.0000x reference)
"""Trainium2 Bass kernel for nn_CC2TBAELoss (data-parallel loss over n=20000).

Strategy: pure data parallelism over 8 NeuronCores (2500 samples each, padded
to 2560 = 20 tiles of 128 samples). Each core streams its shard once from HBM
and produces per-partition partial sums for the 5 loss terms; the host does the
final (tiny) reduction and weighting.

Per 128-sample tile (sample index s in [0,128); R = s//32, u = s%32):
  - mse/contractive/hessian/tangent terms: fused square(+diff)+accumulate on
    ACT/DVE, single pass per element, layout-agnostic.
  - curvature term: per-sample small matmuls on the TensorEngine, packed via
    tile_position 32x32 sub-arrays. All tiles are written by exactly one DMA
    or one compute op (keeps the race detector provably clean):
      dpiT  = DVE 32x32 block-transpose of the natural dpi tile
              -> dpiT[32R + a, 32dd + u] = dpi[s, dd, a]
      covT  = DVE 32x32 block-transpose of the natural cov tile
              -> covT[32R + b, 32a + u] = cov[s, a, b]
      step1 M'_s = cov_s @ dpi_s^T     (covT slice stationary, dpiT moving)
              -> psum_M[32R + a, 8u + dd] = M'_s[a, dd]
      step2 bbt_s = dpi_s @ M'_s       (dpiT slice stationary, M' moving),
            written to PSUM with strided columns (diagonal tile_position --
            off-diagonal col groups crash this HW/toolchain):
              psum_b[32R + j, 32k + u] = bbt_s[j, k]
    so ONE DVE 32x32 block-transpose lands bbt in NATURAL
    samples-on-partitions layout: bbt_sp[s, 32k + j] = bbt_s[j, k].
    All other per-sample tensors load naturally; qv / tangent_vector / npv
    are plain DVE broadcast-multiply + segmented reductions per partition.
"""

import os
import sys

import numpy as np

for _p in ("/opt/trn_rl_repo", "/root/.axon_site/_ro/trn_rl_repo"):
    if os.path.isdir(_p) and _p not in sys.path:
        sys.path.insert(0, _p)

import concourse.bacc as bacc
import concourse.bass as bass
import concourse.tile as tile
from concourse import mybir
from concourse.bass_utils import run_bass_kernel_spmd

F32 = mybir.dt.float32
N_TOTAL = 20000
D = 32
DD = 8
N_CORES = 8
P = 128
PER_CORE_PAD = 2560  # 2500 padded up to a multiple of 128

_CACHE = {}


def _build(n_per_core: int, stage: int = 99) -> bass.Bass:
    assert n_per_core % P == 0
    nt = n_per_core // P
    nc = bacc.Bacc("TRN2")

    shapes = {
        "x_hat": [n_per_core, D],
        "dpi": [n_per_core, DD, D],
        "model_projection": [n_per_core, D, D],
        "decoder_hessian": [n_per_core, D, DD, DD],
        "encoder_hessian": [n_per_core, DD, D, D],
        "x": [n_per_core, D],
        "ambient_drift": [n_per_core, D],
        "ambient_cov": [n_per_core, D, D],
        "observed_projection": [n_per_core, D, D],
    }
    ins = {
        k: nc.dram_tensor(k, shp, F32, kind="ExternalInput").ap()
        for k, shp in shapes.items()
    }
    out = nc.dram_tensor("out", [P, 8], F32, kind="ExternalOutput").ap()

    AX = mybir.AxisListType
    OP = mybir.AluOpType
    ACTF = mybir.ActivationFunctionType

    with tile.TileContext(nc) as tc:
        with (
            tc.tile_pool(name="big", bufs=4) as big,
            tc.tile_pool(name="hp", bufs=4) as hp,
            tc.tile_pool(name="mid", bufs=3) as mid,
            tc.tile_pool(name="small", bufs=4) as small,
            tc.tile_pool(name="accp", bufs=1) as accp,
            tc.tile_pool(name="psum", bufs=2, space="PSUM") as psp,
        ):
            zbias = accp.tile([P, 1], F32, tag="zbias")
            nc.vector.memset(zbias, 0.0)
            acc_mse = accp.tile([P, nt], F32, tag="acc_mse")
            acc_dpi = accp.tile([P, nt], F32, tag="acc_dpi")
            acc_enc = accp.tile([P, 2 * nt], F32, tag="acc_enc")
            acc_tang = accp.tile([P, nt], F32, tag="acc_tang")
            acc_curv = accp.tile([P, nt], F32, tag="acc_curv")

            def b_stage(pv):
                dpiT_v = pv["dpiT_v"]
                MT_t = pv["MT_t"]
                # step2: bbt_s = dpi_s @ M'_s -> psum_b[32R + j, 32k + u]
                psum_b = psp.tile([P, 256], F32, tag="pb")
                nc.vector.memset(psum_b, 0.0)
                pbv = psum_b.rearrange("p (k w) -> p k w", k=8)
                for u in range(32):
                    for R in range(4):
                        nc.tensor.matmul(
                            out=pbv[32 * R : 32 * R + 8, :, u],
                            lhsT=dpiT_v[32 * R : 32 * R + 32, :, u],
                            rhs=MT_t[32 * R : 32 * R + 32, 8 * u : 8 * u + 8],
                            start=True,
                            stop=True,
                            tile_position=(32 * R, 32 * R),
                        )
                bbt_sp = small.tile([P, 256], F32, tag="bbt_sp")
                nc.vector.transpose(out=bbt_sp, in_=psum_b)
                # bbt_sp[s, 32k + j] = bbt_s[j, k]  (s natural = 32R + u)
                pv["bbt_sp"] = bbt_sp

            def c_stage(pv):
                bbt_sp = pv["bbt_sp"]
                H_t = pv["H_t"]
                obs_t = pv["obs_t"]
                dr_t = pv["dr_t"]
                tp = pv["tcol"]
                # qv[i] = sum_{k,j} H[i,k,j] * bbt[j,k]
                H4 = H_t.rearrange("p (i k j) -> p i k j", i=32, k=8)
                bbt_v = bbt_sp.rearrange("p (k j) -> p k j", k=8)[:, :, 0:8]
                bbt_b = bbt_v[:, None, :, :].broadcast_to((P, 32, 8, 8))
                nc.gpsimd.tensor_mul(H4, H4, bbt_b)
                qv_t = small.tile([P, D], F32, tag="qv")
                nc.vector.tensor_reduce(
                    out=qv_t,
                    in_=H_t.rearrange("p (i q) -> p i q", i=32),
                    axis=AX.X,
                    op=OP.add,
                )
                tt = small.tile([P, D], F32, tag="tt")
                nc.vector.scalar_tensor_tensor(
                    out=tt, in0=qv_t, scalar=-0.5, in1=dr_t,
                    op0=OP.mult, op1=OP.add,
                )
                obs3 = obs_t.rearrange("p (r i) -> p r i", r=32)
                t_b = tt[:, None, :].broadcast_to((P, 32, 32))
                nc.gpsimd.tensor_mul(obs3, obs3, t_b)
                Pt_t = small.tile([P, D], F32, tag="Pt")
                nc.vector.tensor_reduce(out=Pt_t, in_=obs3, axis=AX.X, op=OP.add)
                npv_t = small.tile([P, D], F32, tag="npv")
                nc.vector.scalar_tensor_tensor(
                    out=npv_t, in0=Pt_t, scalar=-1.0, in1=tt,
                    op0=OP.mult, op1=OP.add,
                )
                scr2 = small.tile([P, D], F32, tag="scr2")
                nc.vector.tensor_mul(scr2, npv_t, npv_t)
                nc.vector.tensor_reduce(
                    out=acc_curv[:, tp : tp + 1], in_=scr2, axis=AX.X, op=OP.add
                )

            prev = None
            prev2 = None
            for t in range(nt):
                sl = slice(t * P, (t + 1) * P)

                # ------------- DMA loads (PE-feeding tiles first) -------------
                dpi_t = small.tile([P, DD * D], F32, tag="dpi")
                nc.sync.dma_start(
                    out=dpi_t, in_=ins["dpi"][sl].rearrange("s dd a -> s (dd a)")
                )
                cov_t = mid.tile([P, D * D], F32, tag="cov")
                nc.sync.dma_start(
                    out=cov_t,
                    in_=ins["ambient_cov"][sl].rearrange("s a b -> s (a b)"),
                )
                H_t = hp.tile([P, D * DD * DD], F32, tag="H")
                nc.scalar.dma_start(
                    out=H_t,
                    in_=ins["decoder_hessian"][sl].rearrange("s i k j -> s (i k j)"),
                )
                obs_t = mid.tile([P, D * D], F32, tag="obs")
                nc.scalar.dma_start(
                    out=obs_t,
                    in_=ins["observed_projection"][sl].rearrange("s i j -> s (i j)"),
                )
                mdl_t = mid.tile([P, D * D], F32, tag="mdl")
                nc.scalar.dma_start(
                    out=mdl_t,
                    in_=ins["model_projection"][sl].rearrange("s i j -> s (i j)"),
                )
                dr_t = small.tile([P, D], F32, tag="dr")
                nc.gpsimd.dma_start(out=dr_t, in_=ins["ambient_drift"][sl])
                xh_t = small.tile([P, D], F32, tag="xh")
                nc.gpsimd.dma_start(out=xh_t, in_=ins["x_hat"][sl])
                x_t = small.tile([P, D], F32, tag="x")
                nc.gpsimd.dma_start(out=x_t, in_=ins["x"][sl])
                enc_src = ins["encoder_hessian"][sl].rearrange("s a b c -> s (a b c)")
                ECH = 4096
                enc_chunks = []
                for ec in range(2):
                    ech_t = big.tile([P, ECH], F32, tag="enc")
                    nc.sync.dma_start(
                        out=ech_t, in_=enc_src[:, ec * ECH : (ec + 1) * ECH]
                    )
                    enc_chunks.append(ech_t)

                # curvature-pipeline transposes first: they only need the
                # dpi/cov DMAs (issued first), so DVE/PE can start early.
                if stage < 2:
                    continue
                dpiT_t = small.tile([P, DD * D], F32, tag="dpiT")
                nc.vector.transpose(out=dpiT_t, in_=dpi_t)
                # dpiT_t[32R + a, 32dd + u] = dpi[32R + u, dd, a]
                covT_t = mid.tile([P, D * D], F32, tag="covT")
                nc.vector.transpose(out=covT_t, in_=cov_t)
                # covT_t[32R + b, 32a + u] = cov[32R + u, a, b]
                nc.scalar.activation(
                    out=dpi_t,
                    in_=dpi_t,
                    func=ACTF.Square,
                    bias=zbias,
                    accum_out=acc_dpi[:, t : t + 1],
                )
                dpiT_v = dpiT_t.rearrange("p (dd u) -> p dd u", dd=8)
                covT_v = covT_t.rearrange("p (a u) -> p a u", a=32)

                # qv chain for tile t-2: all inputs ready, never stalls DVE.
                if prev2 is not None:
                    c_stage(prev2)
                    prev2 = None

                # ------------- streaming reduction terms -------------
                diff = small.tile([P, D], F32, tag="diff")
                nc.vector.tensor_sub(diff, xh_t, x_t)
                scr = small.tile([P, D], F32, tag="scr")
                nc.vector.tensor_mul(scr, diff, diff)
                nc.vector.tensor_reduce(
                    out=acc_mse[:, t : t + 1], in_=scr, axis=AX.X, op=OP.add
                )
                for ec in range(2):
                    nc.scalar.activation(
                        out=enc_chunks[ec],
                        in_=enc_chunks[ec],
                        func=ACTF.Square,
                        bias=zbias,
                        accum_out=acc_enc[:, 2 * t + ec : 2 * t + ec + 1],
                    )
                nc.gpsimd.tensor_sub(mdl_t, mdl_t, obs_t)
                nc.scalar.activation(
                    out=mdl_t,
                    in_=mdl_t,
                    func=ACTF.Square,
                    bias=zbias,
                    accum_out=acc_tang[:, t : t + 1],
                )

                if stage < 3:
                    continue
                # B-stage for tile t-1: PE2 + bbt PSUM->SBUF transpose.
                if prev is not None:
                    b_stage(prev)
                    prev2 = prev
                    prev = None

                # step1: M'_s = cov_s @ dpi_s^T  -> psum_M[32R + a, 8u + dd]
                psum_M = psp.tile([P, 256], F32, tag="pm")
                for u in range(32):
                    for R in range(4):
                        nc.tensor.matmul(
                            out=psum_M[32 * R : 32 * R + 32, 8 * u : 8 * u + 8],
                            lhsT=covT_v[32 * R : 32 * R + 32, :, u],
                            rhs=dpiT_v[32 * R : 32 * R + 32, :, u],
                            start=True,
                            stop=True,
                            tile_position=(32 * R, 32 * R),
                        )
                MT_t = small.tile([P, 256], F32, tag="MT")
                nc.scalar.copy(out=MT_t, in_=psum_M)
                prev = dict(
                    dpiT_v=dpiT_v, MT_t=MT_t, H_t=H_t, obs_t=obs_t,
                    dr_t=dr_t, tcol=t,
                )

            if prev2 is not None:
                c_stage(prev2)
                prev2 = None
            if prev is not None:
                b_stage(prev)
                c_stage(prev)
                prev = None

            # ------------- final packing -------------
            outsb = accp.tile([P, 8], F32, tag="outsb")
            nc.vector.memset(outsb, 0.0)
            if stage < 5:
                nc.vector.memset(acc_curv, 0.0)
            if stage < 2:
                nc.vector.memset(acc_dpi, 0.0)
            for j, acc in enumerate([acc_mse, acc_dpi, acc_enc, acc_tang, acc_curv]):
                nc.vector.tensor_reduce(
                    out=outsb[:, j : j + 1], in_=acc, axis=AX.X, op=OP.add
                )
            nc.sync.dma_start(out=out, in_=outsb)

    nc.finalize()
    return nc


def _get_nc(n_per_core: int) -> bass.Bass:
    if n_per_core not in _CACHE:
        _CACHE[n_per_core] = _build(n_per_core)
    return _CACHE[n_per_core]


def _make_in_maps(inputs: dict, per: int, nper: int) -> list[dict]:
    in_maps = []
    for ci in range(N_CORES):
        m = {}
        for k, arr in inputs.items():
            a = np.asarray(arr)[ci * per : (ci + 1) * per].astype(
                np.float32, copy=False
            )
            if nper > per:
                pad = np.zeros((nper - per,) + a.shape[1:], np.float32)
                a = np.concatenate([a, pad], axis=0)
            m[k] = np.ascontiguousarray(a)
        in_maps.append(m)
    return in_maps


def _combine(results, n_total: int) -> np.ndarray:
    parts = np.stack([r["out"] for r in results]).astype(np.float64)
    s = parts.sum(axis=(0, 1))
    loss = s[0] / (n_total * D) + (s[1] + s[2] + s[3]) / n_total + s[4]
    return np.array(loss, dtype=np.float32)


def run(inputs: dict, trace: bool = False):
    """Returns (loss, exec_time_ns or None). Used by kernel() and test.py."""
    n_total = np.asarray(inputs["x_hat"]).shape[0]
    per = n_total // N_CORES
    nper = ((per + P - 1) // P) * P
    nc = _get_nc(nper)
    in_maps = _make_in_maps(inputs, per, nper)
    res = run_bass_kernel_spmd(
        nc, in_maps, core_ids=list(range(N_CORES)), trace=trace
    )
    return _combine(res.results, n_total), res.exec_time_ns


def kernel(**inputs) -> np.ndarray:
    loss, _ = run(inputs)
    return loss



# revision 2
# speedup vs baseline: 1.0109x; 1.0109x over previous
"""Trainium2 Bass kernel for nn_CC2TBAELoss (data-parallel loss over n=20000).

v4: host-concatenated inputs -> two large sync-queue DMAs per 128-sample
tile, 3-deep software pipeline, and a HAND-SCHEDULED static order via
tile_wait_until floors (21us/tile period = measured HBM streaming rate).
This stops the Tile scheduler from parking the PE-feeding transposes
behind the long c_stage chain, which otherwise inflates the steady-state
period to ~31us/tile (v3 measured 630us; pure-DMA roofline is 418us).

Pipeline (iteration/block t):
  T0+0.0  rest(t), enc(t) DMA triggers        [sync]
  T0+0.2  MT copy(t-1) PSUM->SBUF             [scalar]
  T0+0.3  psum_b memset(t-1)                  [DVE]
  T0+0.5  bbt transpose(t-2)                  [DVE]
  T0+0.7  step2(t-1): bbt = dpi @ M'          [PE]
  T0+0.8  enc square(t-1) (data landed ~T0)   [scalar/ACT]
  T0+1..12 c_stage(t-2): qv -> tv -> npv -> curv  [DVE+gpsimd halves]
  T0+8.4  dpiT(t), covT(t) (rest(t) landed)   [DVE]
  T0+9.5  dpi square(t)                       [scalar]
  T0+10.2 step1(t): M' = cov @ dpi^T          [PE]
  T0+12.2 mse diff+square(t), tangent diff+square(t)  [DVE+scalar]
"""

import os
import sys

import numpy as np

for _p in ("/opt/trn_rl_repo", "/root/.axon_site/_ro/trn_rl_repo"):
    if os.path.isdir(_p) and _p not in sys.path:
        sys.path.insert(0, _p)

import concourse.bacc as bacc
import concourse.bass as bass
import concourse.tile as tile
from concourse import mybir
from concourse.bass_utils import run_bass_kernel_spmd

F32 = mybir.dt.float32
N_TOTAL = 20000
D = 32
DD = 8
N_CORES = 8
P = 128
W = 13664
WE = 8192
WR = W - WE  # 5472
O_XH, O_X, O_DR, O_DPI, O_COV, O_OBS, O_MDL, O_H = 0, 32, 64, 96, 352, 1376, 2400, 3424
HS = 1280  # H elementwise split: DVE does i<20, gpsimd i>=20
OS = 640   # obs elementwise split: DVE rows<20, gpsimd rows>=20
PD = 21.0  # schedule period per tile, us

_CACHE = {}


def _build(n_per_core: int) -> bass.Bass:
    assert n_per_core % P == 0
    nt = n_per_core // P
    nc = bacc.Bacc("TRN2")

    rest_in = nc.dram_tensor("rest", [n_per_core, WR], F32, kind="ExternalInput").ap()
    enc_in = nc.dram_tensor("enc", [n_per_core, WE], F32, kind="ExternalInput").ap()
    out = nc.dram_tensor("out", [P, 8], F32, kind="ExternalOutput").ap()

    AX = mybir.AxisListType
    OP = mybir.AluOpType
    ACTF = mybir.ActivationFunctionType

    with tile.TileContext(nc) as tc:
        with (
            tc.tile_pool(name="encp", bufs=5) as encp,
            tc.tile_pool(name="restp", bufs=5) as restp,
            tc.tile_pool(name="sm4", bufs=4) as sm4,
            tc.tile_pool(name="sm2", bufs=2) as sm2,
            tc.tile_pool(name="accp", bufs=1) as accp,
            tc.tile_pool(name="psum", bufs=2, space="PSUM") as psp,
        ):
            zbias = accp.tile([P, 1], F32, tag="zbias")
            nc.vector.memset(zbias, 0.0)
            acc_mse = accp.tile([P, nt], F32, tag="acc_mse")
            acc_dpi = accp.tile([P, nt], F32, tag="acc_dpi")
            acc_enc = accp.tile([P, 2 * nt], F32, tag="acc_enc")
            acc_tang = accp.tile([P, nt], F32, tag="acc_tang")
            acc_curv = accp.tile([P, nt], F32, tag="acc_curv")

            stage = {}

            for t in range(nt + 2):
                if t < nt:
                    T0 = t * PD
                else:
                    T0 = (nt - 1) * PD + (t - (nt - 1)) * 12.0

                def w(us):
                    return tc.tile_wait_until((T0 + us) / 1000.0)

                # ---------------- DMA triggers (sync engine only) --------
                if t < nt:
                    sl = slice(t * P, (t + 1) * P)
                    with w(0.0):
                        rest_t = restp.tile([P, WR], F32, tag="rest")
                        nc.sync.dma_start(out=rest_t, in_=rest_in[sl])
                    with w(0.1):
                        enc_a = encp.tile([P, WE // 2], F32, tag="enc")
                        nc.sync.dma_start(out=enc_a, in_=enc_in[sl, 0 : WE // 2])
                    with w(0.2):
                        enc_b = encp.tile([P, WE // 2], F32, tag="enc")
                        nc.sync.dma_start(out=enc_b, in_=enc_in[sl, WE // 2 : WE])
                    stage[t] = dict(rest_t=rest_t, enc_a=enc_a, enc_b=enc_b)

                # ---- scalar: enc square(t-1) (data landed ~T0) ----------
                mt_fl, enc_fl, st2_fl = (7.5, 0.2, 8.0) if t < nt - 3 else (0.2, 1.0, 0.8)
                if 1 <= t <= nt:
                    pv = stage[t - 1]
                    with w(enc_fl):
                        nc.scalar.activation(
                            out=pv["enc_a"], in_=pv["enc_a"], func=ACTF.Square,
                            bias=zbias,
                            accum_out=acc_enc[:, 2 * t - 2 : 2 * t - 1],
                        )
                    with w(enc_fl + 0.2):
                        nc.scalar.activation(
                            out=pv["enc_b"], in_=pv["enc_b"], func=ACTF.Square,
                            bias=zbias,
                            accum_out=acc_enc[:, 2 * t - 1 : 2 * t],
                        )
                    with w(mt_fl):
                        MT_t = sm2.tile([P, 256], F32, tag="MT")
                        nc.scalar.copy(out=MT_t, in_=pv["psum_M"])
                    pv["MT_t"] = MT_t
                    with w(0.3):
                        psum_b = psp.tile([P, 256], F32, tag="pb")
                        nc.vector.memset(psum_b, 0.0)
                    pv["psum_b"] = psum_b

                # ---- DVE: bbt transpose(t-2) ----------------------------
                drain = t >= nt - 1
                if t >= 2:
                    pv = stage[t - 2]
                    with w(16.2 if drain else 0.5):
                        bbt_sp = sm2.tile([P, 256], F32, tag="bbt_sp", bufs=1)
                        nc.vector.transpose(out=bbt_sp, in_=pv["psum_b"])
                    pv["bbt_sp"] = bbt_sp

                # ---- PE: step2(t-1): bbt = dpi @ M' ---------------------
                if 1 <= t <= nt:
                    pv = stage[t - 1]
                    dpiT_v = pv["dpiT_v"]
                    MT_t = pv["MT_t"]
                    pbv = pv["psum_b"].rearrange("p (k w) -> p k w", k=8)
                    with w(st2_fl):
                        for u in range(32):
                            for R in range(4):
                                nc.tensor.matmul(
                                    out=pbv[32 * R : 32 * R + 8, :, u],
                                    lhsT=dpiT_v[32 * R : 32 * R + 32, :, u],
                                    rhs=MT_t[32 * R : 32 * R + 32, 8 * u : 8 * u + 8],
                                    start=True,
                                    stop=True,
                                    tile_position=(32 * R, 32 * R),
                                )

                # ---- c_stage(t-2): qv -> tangent -> npv -> curv ---------
                if t >= 2:
                    pv = stage.pop(t - 2)
                    rest_p = pv["rest_t"]
                    bbt_sp = pv["bbt_sp"]
                    tp = t - 2
                    H4 = rest_p[:, O_H : O_H + D * DD * DD].rearrange(
                        "p (i k j) -> p i k j", i=32, k=8
                    )
                    bbt_v = bbt_sp.rearrange("p (k j) -> p k j", k=8)[:, :, 0:8]
                    bbt_b = bbt_v[:, None, :, :].broadcast_to((P, 32, 8, 8))
                    nh = 32 if drain else HS // 64
                    cf = 16.5 if drain else 0.0
                    with w(cf + 1.0):
                        if nh < 32:
                            nc.gpsimd.tensor_mul(
                                H4[:, nh:32], H4[:, nh:32], bbt_b[:, nh:32]
                            )
                        nc.vector.tensor_mul(
                            H4[:, 0:nh], H4[:, 0:nh], bbt_b[:, 0:nh]
                        )
                    with w(cf + 4.0):
                        qv_t = sm2.tile([P, D], F32, tag="qv", bufs=1)
                        nc.vector.tensor_reduce(
                            out=qv_t,
                            in_=rest_p[:, O_H : O_H + D * DD * DD].rearrange(
                                "p (i q) -> p i q", i=32
                            ),
                            axis=AX.X,
                            op=OP.add,
                        )
                    with w(cf + 6.3):
                        tt = sm2.tile([P, D], F32, tag="tt", bufs=1)
                        nc.vector.scalar_tensor_tensor(
                            out=tt, in0=qv_t, scalar=-0.5,
                            in1=rest_p[:, O_DR : O_DR + D],
                            op0=OP.mult, op1=OP.add,
                        )
                    obs3 = rest_p[:, O_OBS : O_OBS + D * D].rearrange(
                        "p (r i) -> p r i", r=32
                    )
                    t_b = tt[:, None, :].broadcast_to((P, 32, 32))
                    no = 32 if drain else OS // 32
                    with w(cf + 6.5):
                        if no < 32:
                            nc.gpsimd.tensor_mul(
                                obs3[:, no:32], obs3[:, no:32], t_b[:, no:32]
                            )
                        nc.vector.tensor_mul(
                            obs3[:, 0:no], obs3[:, 0:no], t_b[:, 0:no]
                        )
                    with w(cf + 10.2):
                        Pt_t = sm2.tile([P, D], F32, tag="Pt", bufs=1)
                        nc.vector.tensor_reduce(
                            out=Pt_t, in_=obs3, axis=AX.X, op=OP.add
                        )
                    with w(cf + 11.5):
                        npv_t = sm2.tile([P, D], F32, tag="npv", bufs=1)
                        nc.vector.scalar_tensor_tensor(
                            out=npv_t, in0=Pt_t, scalar=-1.0, in1=tt,
                            op0=OP.mult, op1=OP.add,
                        )
                    with w(cf + 11.7):
                        scr2 = sm2.tile([P, D], F32, tag="scr2", bufs=1)
                        nc.vector.tensor_mul(scr2, npv_t, npv_t)
                    with w(cf + 11.9):
                        nc.vector.tensor_reduce(
                            out=acc_curv[:, tp : tp + 1], in_=scr2,
                            axis=AX.X, op=OP.add,
                        )

                # ---- DVE: transposes for tile t (feed PE step1) ---------
                if t < nt:
                    pv = stage[t]
                    rest_t = pv["rest_t"]
                    with w(8.4):
                        dpiT_t = sm4.tile([P, DD * D], F32, tag="dpiT")
                        nc.vector.transpose(
                            out=dpiT_t, in_=rest_t[:, O_DPI : O_DPI + DD * D]
                        )
                    with w(8.9):
                        covT_t = sm2.tile([P, D * D], F32, tag="covT")
                        nc.vector.transpose(
                            out=covT_t, in_=rest_t[:, O_COV : O_COV + D * D]
                        )
                    pv["dpiT_v"] = dpiT_t.rearrange("p (dd u) -> p dd u", dd=8)
                    pv["covT_v"] = covT_t.rearrange("p (a u) -> p a u", a=32)
                    with w(9.5):
                        nc.scalar.activation(
                            out=rest_t[:, O_DPI : O_DPI + DD * D],
                            in_=rest_t[:, O_DPI : O_DPI + DD * D],
                            func=ACTF.Square, bias=zbias,
                            accum_out=acc_dpi[:, t : t + 1],
                        )

                # ---- PE: step1(t): M' = cov @ dpi^T ---------------------
                if t < nt:
                    pv = stage[t]
                    dpiT_v = pv["dpiT_v"]
                    covT_v = pv["covT_v"]
                    with w(10.2):
                        psum_M = psp.tile([P, 256], F32, tag="pm")
                        for u in range(32):
                            for R in range(4):
                                nc.tensor.matmul(
                                    out=psum_M[
                                        32 * R : 32 * R + 32, 8 * u : 8 * u + 8
                                    ],
                                    lhsT=covT_v[32 * R : 32 * R + 32, :, u],
                                    rhs=dpiT_v[32 * R : 32 * R + 32, :, u],
                                    start=True,
                                    stop=True,
                                    tile_position=(32 * R, 32 * R),
                                )
                    pv["psum_M"] = psum_M

                # ---- mse + tangent terms for tile t ---------------------
                if t < nt:
                    pv = stage[t]
                    rest_t = pv["rest_t"]
                    with w(12.2):
                        xsl = rest_t[:, O_X : O_X + D]
                        nc.vector.scalar_tensor_tensor(
                            out=xsl, in0=xsl, scalar=-1.0,
                            in1=rest_t[:, O_XH : O_XH + D],
                            op0=OP.mult, op1=OP.add,
                        )
                    with w(12.5):
                        nc.scalar.activation(
                            out=xsl, in_=xsl, func=ACTF.Square,
                            bias=zbias, accum_out=acc_mse[:, t : t + 1],
                        )
                    with w(13.5):
                        msl = rest_t[:, O_MDL : O_MDL + D * D]
                        nc.vector.scalar_tensor_tensor(
                            out=msl, in0=rest_t[:, O_OBS : O_OBS + D * D],
                            scalar=-1.0, in1=msl,
                            op0=OP.mult, op1=OP.add,
                        )
                    with w(13.8):
                        nc.scalar.activation(
                            out=msl, in_=msl, func=ACTF.Square,
                            bias=zbias, accum_out=acc_tang[:, t : t + 1],
                        )

            # ---------------- final packing --------------------------
            with tc.tile_wait_until(((nt - 1) * PD + 60.0) / 1000.0):
                outsb = accp.tile([P, 8], F32, tag="outsb")
                nc.vector.memset(outsb, 0.0)
                for j, acc in enumerate(
                    [acc_mse, acc_dpi, acc_enc, acc_tang, acc_curv]
                ):
                    nc.vector.tensor_reduce(
                        out=outsb[:, j : j + 1], in_=acc, axis=AX.X, op=OP.add
                    )
                nc.sync.dma_start(out=out, in_=outsb)

    nc.finalize()
    return nc


def _get_nc(n_per_core: int) -> bass.Bass:
    if n_per_core not in _CACHE:
        _CACHE[n_per_core] = _build(n_per_core)
    return _CACHE[n_per_core]


def _make_in_maps(inputs: dict, per: int, nper: int) -> list[dict]:
    """Concatenate per-sample data into [rest | enc] arrays per core."""
    order = [
        ("x_hat", 32), ("x", 32), ("ambient_drift", 32), ("dpi", 256),
        ("ambient_cov", 1024), ("observed_projection", 1024),
        ("model_projection", 1024), ("decoder_hessian", 2048),
    ]
    in_maps = []
    for ci in range(N_CORES):
        sl = slice(ci * per, (ci + 1) * per)
        rest = np.zeros((nper, WR), np.float32)
        off = 0
        for k, w_ in order:
            a = np.asarray(inputs[k])[sl]
            rest[:per, off : off + w_] = a.reshape(per, w_)
            off += w_
        assert off == WR
        enc = np.zeros((nper, WE), np.float32)
        enc[:per] = np.asarray(inputs["encoder_hessian"])[sl].reshape(per, WE)
        in_maps.append({"rest": rest, "enc": enc})
    return in_maps


def _combine(results, n_total: int) -> np.ndarray:
    parts = np.stack([r["out"] for r in results]).astype(np.float64)
    s = parts.sum(axis=(0, 1))
    loss = s[0] / (n_total * D) + (s[1] + s[2] + s[3]) / n_total + s[4]
    return np.array(loss, dtype=np.float32)


def run(inputs: dict, trace: bool = False):
    """Returns (loss, exec_time_ns or None). Used by kernel() and test.py."""
    n_total = np.asarray(inputs["x_hat"]).shape[0]
    per = n_total // N_CORES
    nper = ((per + P - 1) // P) * P
    nc = _get_nc(nper)
    in_maps = _make_in_maps(inputs, per, nper)
    res = run_bass_kernel_spmd(
        nc, in_maps, core_ids=list(range(N_CORES)), trace=trace
    )
    return _combine(res.results, n_total), res.exec_time_ns


def kernel(**inputs) -> np.ndarray:
    loss, _ = run(inputs)
    return loss


# revision 3
# speedup vs baseline: 1.0190x; 1.0080x over previous
"""Trainium2 Bass kernel for nn_CC2TBAELoss (data-parallel loss over n=20000).

v4: host-concatenated inputs -> two large sync-queue DMAs per 128-sample
tile, 3-deep software pipeline, and a HAND-SCHEDULED static order via
tile_wait_until floors (21us/tile period = measured HBM streaming rate).
This stops the Tile scheduler from parking the PE-feeding transposes
behind the long c_stage chain, which otherwise inflates the steady-state
period to ~31us/tile (v3 measured 630us; pure-DMA roofline is 418us).

Pipeline (iteration/block t):
  T0+0.0  rest(t), enc(t) DMA triggers        [sync]
  T0+0.2  MT copy(t-1) PSUM->SBUF             [scalar]
  T0+0.3  psum_b memset(t-1)                  [DVE]
  T0+0.5  bbt transpose(t-2)                  [DVE]
  T0+0.7  step2(t-1): bbt = dpi @ M'          [PE]
  T0+0.8  enc square(t-1) (data landed ~T0)   [scalar/ACT]
  T0+1..12 c_stage(t-2): qv -> tv -> npv -> curv  [DVE+gpsimd halves]
  T0+8.4  dpiT(t), covT(t) (rest(t) landed)   [DVE]
  T0+9.5  dpi square(t)                       [scalar]
  T0+10.2 step1(t): M' = cov @ dpi^T          [PE]
  T0+12.2 mse diff+square(t), tangent diff+square(t)  [DVE+scalar]
"""

import os
import sys

import numpy as np

for _p in ("/opt/trn_rl_repo", "/root/.axon_site/_ro/trn_rl_repo"):
    if os.path.isdir(_p) and _p not in sys.path:
        sys.path.insert(0, _p)

import concourse.bacc as bacc
import concourse.bass as bass
import concourse.tile as tile
from concourse import mybir
from concourse.bass_utils import run_bass_kernel_spmd

F32 = mybir.dt.float32
N_TOTAL = 20000
D = 32
DD = 8
N_CORES = 8
P = 128
W = 13664
WE = 8192
WR = W - WE  # 5472
O_XH, O_X, O_DR, O_DPI, O_COV, O_OBS, O_MDL, O_H = 0, 32, 64, 96, 352, 1376, 2400, 3424
HS = 1280  # H elementwise split: DVE does i<20, gpsimd i>=20
OS = 640   # obs elementwise split: DVE rows<20, gpsimd rows>=20
PD = 21.0  # schedule period per tile, us

_CACHE = {}


def _build(n_per_core: int) -> bass.Bass:
    assert n_per_core % P == 0
    nt = n_per_core // P
    nc = bacc.Bacc("TRN2")

    rest_in = nc.dram_tensor("rest", [n_per_core, WR], F32, kind="ExternalInput").ap()
    enc_in = nc.dram_tensor("enc", [n_per_core, WE], F32, kind="ExternalInput").ap()
    out = nc.dram_tensor("out", [P, 8], F32, kind="ExternalOutput").ap()

    AX = mybir.AxisListType
    OP = mybir.AluOpType
    ACTF = mybir.ActivationFunctionType

    with tile.TileContext(nc) as tc:
        with (
            tc.tile_pool(name="encp", bufs=5) as encp,
            tc.tile_pool(name="restp", bufs=5) as restp,
            tc.tile_pool(name="sm4", bufs=4) as sm4,
            tc.tile_pool(name="sm2", bufs=2) as sm2,
            tc.tile_pool(name="accp", bufs=1) as accp,
            tc.tile_pool(name="psum", bufs=4, space="PSUM") as psp,
        ):
            zbias = accp.tile([P, 1], F32, tag="zbias")
            nc.vector.memset(zbias, 0.0)
            acc_mse = accp.tile([P, nt], F32, tag="acc_mse")
            acc_dpi = accp.tile([P, nt], F32, tag="acc_dpi")
            acc_enc = accp.tile([P, 2 * nt], F32, tag="acc_enc")
            acc_tang = accp.tile([P, nt], F32, tag="acc_tang")
            acc_curv = accp.tile([P, nt], F32, tag="acc_curv")

            stage = {}

            for t in range(nt + 2):
                if t < nt:
                    T0 = t * PD
                else:
                    T0 = (nt - 1) * PD + (t - (nt - 1)) * 12.0

                def w(us):
                    return tc.tile_wait_until((T0 + us) / 1000.0)

                # ---------------- DMA triggers (sync engine only) --------
                if t < nt:
                    sl = slice(t * P, (t + 1) * P)
                    with w(0.0):
                        rest_t = restp.tile([P, WR], F32, tag="rest")
                        nc.sync.dma_start(out=rest_t, in_=rest_in[sl])
                    with w(0.1):
                        enc_a = encp.tile([P, WE // 2], F32, tag="enc")
                        nc.sync.dma_start(out=enc_a, in_=enc_in[sl, 0 : WE // 2])
                    with w(0.2):
                        enc_b = encp.tile([P, WE // 2], F32, tag="enc")
                        nc.sync.dma_start(out=enc_b, in_=enc_in[sl, WE // 2 : WE])
                    stage[t] = dict(rest_t=rest_t, enc_a=enc_a, enc_b=enc_b)

                # ---- scalar: enc square(t-1) (data landed ~T0) ----------
                mt_fl, enc_fl, st2_fl = (0.2, 1.0, 0.8)
                if 1 <= t <= nt:
                    pv = stage[t - 1]
                    with w(enc_fl):
                        nc.scalar.activation(
                            out=pv["enc_a"], in_=pv["enc_a"], func=ACTF.Square,
                            bias=zbias,
                            accum_out=acc_enc[:, 2 * t - 2 : 2 * t - 1],
                        )
                    with w(enc_fl + 0.2):
                        nc.scalar.activation(
                            out=pv["enc_b"], in_=pv["enc_b"], func=ACTF.Square,
                            bias=zbias,
                            accum_out=acc_enc[:, 2 * t - 1 : 2 * t],
                        )
                    with w(mt_fl):
                        MT_t = sm2.tile([P, 256], F32, tag="MT")
                        nc.scalar.copy(out=MT_t, in_=pv["psum_M"])
                    pv["MT_t"] = MT_t
                    with w(0.3):
                        psum_b = psp.tile([P, 256], F32, tag="pb")
                        nc.vector.memset(psum_b, 0.0)
                    pv["psum_b"] = psum_b

                # ---- DVE: bbt transpose(t-2) ----------------------------
                drain = t >= nt - 1
                if t >= 2:
                    pv = stage[t - 2]
                    with w(16.2 if drain else 0.5):
                        bbt_sp = sm2.tile([P, 256], F32, tag="bbt_sp", bufs=1)
                        nc.vector.transpose(out=bbt_sp, in_=pv["psum_b"])
                    pv["bbt_sp"] = bbt_sp

                # ---- PE: step2(t-1): bbt = dpi @ M' ---------------------
                if 1 <= t <= nt:
                    pv = stage[t - 1]
                    dpiT_v = pv["dpiT_v"]
                    MT_t = pv["MT_t"]
                    pbv = pv["psum_b"].rearrange("p (k w) -> p k w", k=8)
                    with w(st2_fl):
                        for u in range(32):
                            for R in range(4):
                                nc.tensor.matmul(
                                    out=pbv[32 * R : 32 * R + 8, :, u],
                                    lhsT=dpiT_v[32 * R : 32 * R + 32, :, u],
                                    rhs=MT_t[32 * R : 32 * R + 32, 8 * u : 8 * u + 8],
                                    start=True,
                                    stop=True,
                                    tile_position=(32 * R, 32 * R),
                                )

                # ---- c_stage(t-2): qv -> tangent -> npv -> curv ---------
                if t >= 2:
                    pv = stage.pop(t - 2)
                    rest_p = pv["rest_t"]
                    bbt_sp = pv["bbt_sp"]
                    tp = t - 2
                    H4 = rest_p[:, O_H : O_H + D * DD * DD].rearrange(
                        "p (i k j) -> p i k j", i=32, k=8
                    )
                    bbt_v = bbt_sp.rearrange("p (k j) -> p k j", k=8)[:, :, 0:8]
                    bbt_b = bbt_v[:, None, :, :].broadcast_to((P, 32, 8, 8))
                    nh = 32 if drain else HS // 64
                    cf = 16.5 if drain else 0.0
                    with w(cf + 1.0):
                        if nh < 32:
                            nc.gpsimd.tensor_mul(
                                H4[:, nh:32], H4[:, nh:32], bbt_b[:, nh:32]
                            )
                        nc.vector.tensor_mul(
                            H4[:, 0:nh], H4[:, 0:nh], bbt_b[:, 0:nh]
                        )
                    with w(cf + 4.0):
                        qv_t = sm2.tile([P, D], F32, tag="qv", bufs=1)
                        nc.vector.tensor_reduce(
                            out=qv_t,
                            in_=rest_p[:, O_H : O_H + D * DD * DD].rearrange(
                                "p (i q) -> p i q", i=32
                            ),
                            axis=AX.X,
                            op=OP.add,
                        )
                    with w(cf + 6.3):
                        tt = sm2.tile([P, D], F32, tag="tt", bufs=1)
                        nc.vector.scalar_tensor_tensor(
                            out=tt, in0=qv_t, scalar=-0.5,
                            in1=rest_p[:, O_DR : O_DR + D],
                            op0=OP.mult, op1=OP.add,
                        )
                    obs3 = rest_p[:, O_OBS : O_OBS + D * D].rearrange(
                        "p (r i) -> p r i", r=32
                    )
                    t_b = tt[:, None, :].broadcast_to((P, 32, 32))
                    no = 32 if drain else OS // 32
                    with w(cf + 6.5):
                        if no < 32:
                            nc.gpsimd.tensor_mul(
                                obs3[:, no:32], obs3[:, no:32], t_b[:, no:32]
                            )
                        nc.vector.tensor_mul(
                            obs3[:, 0:no], obs3[:, 0:no], t_b[:, 0:no]
                        )
                    with w(cf + 10.2):
                        Pt_t = sm2.tile([P, D], F32, tag="Pt", bufs=1)
                        nc.vector.tensor_reduce(
                            out=Pt_t, in_=obs3, axis=AX.X, op=OP.add
                        )
                    with w(cf + 11.5):
                        npv_t = sm2.tile([P, D], F32, tag="npv", bufs=1)
                        nc.vector.scalar_tensor_tensor(
                            out=npv_t, in0=Pt_t, scalar=-1.0, in1=tt,
                            op0=OP.mult, op1=OP.add,
                        )
                    with w(cf + 11.7):
                        scr2 = sm2.tile([P, D], F32, tag="scr2", bufs=1)
                        nc.vector.tensor_mul(scr2, npv_t, npv_t)
                    with w(cf + 11.9):
                        nc.vector.tensor_reduce(
                            out=acc_curv[:, tp : tp + 1], in_=scr2,
                            axis=AX.X, op=OP.add,
                        )

                # ---- DVE: transposes for tile t (feed PE step1) ---------
                if t < nt:
                    pv = stage[t]
                    rest_t = pv["rest_t"]
                    with w(8.4):
                        dpiT_t = sm4.tile([P, DD * D], F32, tag="dpiT")
                        nc.vector.transpose(
                            out=dpiT_t, in_=rest_t[:, O_DPI : O_DPI + DD * D]
                        )
                    with w(8.9):
                        covT_t = sm2.tile([P, D * D], F32, tag="covT")
                        nc.vector.transpose(
                            out=covT_t, in_=rest_t[:, O_COV : O_COV + D * D]
                        )
                    pv["dpiT_v"] = dpiT_t.rearrange("p (dd u) -> p dd u", dd=8)
                    pv["covT_v"] = covT_t.rearrange("p (a u) -> p a u", a=32)
                    with w(9.5):
                        nc.scalar.activation(
                            out=rest_t[:, O_DPI : O_DPI + DD * D],
                            in_=rest_t[:, O_DPI : O_DPI + DD * D],
                            func=ACTF.Square, bias=zbias,
                            accum_out=acc_dpi[:, t : t + 1],
                        )

                # ---- PE: step1(t): M' = cov @ dpi^T ---------------------
                if t < nt:
                    pv = stage[t]
                    dpiT_v = pv["dpiT_v"]
                    covT_v = pv["covT_v"]
                    with w(10.2):
                        psum_M = psp.tile([P, 256], F32, tag="pm")
                        for u in range(32):
                            for R in range(4):
                                nc.tensor.matmul(
                                    out=psum_M[
                                        32 * R : 32 * R + 32, 8 * u : 8 * u + 8
                                    ],
                                    lhsT=covT_v[32 * R : 32 * R + 32, :, u],
                                    rhs=dpiT_v[32 * R : 32 * R + 32, :, u],
                                    start=True,
                                    stop=True,
                                    tile_position=(32 * R, 32 * R),
                                )
                    pv["psum_M"] = psum_M

                # ---- mse + tangent terms for tile t ---------------------
                if t < nt:
                    pv = stage[t]
                    rest_t = pv["rest_t"]
                    with w(12.2):
                        xsl = rest_t[:, O_X : O_X + D]
                        nc.vector.scalar_tensor_tensor(
                            out=xsl, in0=xsl, scalar=-1.0,
                            in1=rest_t[:, O_XH : O_XH + D],
                            op0=OP.mult, op1=OP.add,
                        )
                    with w(12.5):
                        nc.scalar.activation(
                            out=xsl, in_=xsl, func=ACTF.Square,
                            bias=zbias, accum_out=acc_mse[:, t : t + 1],
                        )
                    with w(13.5):
                        msl = rest_t[:, O_MDL : O_MDL + D * D]
                        nc.vector.scalar_tensor_tensor(
                            out=msl, in0=rest_t[:, O_OBS : O_OBS + D * D],
                            scalar=-1.0, in1=msl,
                            op0=OP.mult, op1=OP.add,
                        )
                    with w(13.8):
                        nc.scalar.activation(
                            out=msl, in_=msl, func=ACTF.Square,
                            bias=zbias, accum_out=acc_tang[:, t : t + 1],
                        )

            # ---------------- final packing --------------------------
            with tc.tile_wait_until(((nt - 1) * PD + 60.0) / 1000.0):
                outsb = accp.tile([P, 8], F32, tag="outsb")
                nc.vector.memset(outsb, 0.0)
                for j, acc in enumerate(
                    [acc_mse, acc_dpi, acc_enc, acc_tang, acc_curv]
                ):
                    nc.vector.tensor_reduce(
                        out=outsb[:, j : j + 1], in_=acc, axis=AX.X, op=OP.add
                    )
                nc.sync.dma_start(out=out, in_=outsb)

    nc.finalize()
    return nc


def _get_nc(n_per_core: int) -> bass.Bass:
    if n_per_core not in _CACHE:
        _CACHE[n_per_core] = _build(n_per_core)
    return _CACHE[n_per_core]


def _make_in_maps(inputs: dict, per: int, nper: int) -> list[dict]:
    """Concatenate per-sample data into [rest | enc] arrays per core."""
    order = [
        ("x_hat", 32), ("x", 32), ("ambient_drift", 32), ("dpi", 256),
        ("ambient_cov", 1024), ("observed_projection", 1024),
        ("model_projection", 1024), ("decoder_hessian", 2048),
    ]
    in_maps = []
    for ci in range(N_CORES):
        sl = slice(ci * per, (ci + 1) * per)
        rest = np.zeros((nper, WR), np.float32)
        off = 0
        for k, w_ in order:
            a = np.asarray(inputs[k])[sl]
            rest[:per, off : off + w_] = a.reshape(per, w_)
            off += w_
        assert off == WR
        enc = np.zeros((nper, WE), np.float32)
        enc[:per] = np.asarray(inputs["encoder_hessian"])[sl].reshape(per, WE)
        in_maps.append({"rest": rest, "enc": enc})
    return in_maps


def _combine(results, n_total: int) -> np.ndarray:
    parts = np.stack([r["out"] for r in results]).astype(np.float64)
    s = parts.sum(axis=(0, 1))
    loss = s[0] / (n_total * D) + (s[1] + s[2] + s[3]) / n_total + s[4]
    return np.array(loss, dtype=np.float32)


def run(inputs: dict, trace: bool = False):
    """Returns (loss, exec_time_ns or None). Used by kernel() and test.py."""
    n_total = np.asarray(inputs["x_hat"]).shape[0]
    per = n_total // N_CORES
    nper = ((per + P - 1) // P) * P
    nc = _get_nc(nper)
    in_maps = _make_in_maps(inputs, per, nper)
    res = run_bass_kernel_spmd(
        nc, in_maps, core_ids=list(range(N_CORES)), trace=trace
    )
    return _combine(res.results, n_total), res.exec_time_ns


def kernel(**inputs) -> np.ndarray:
    loss, _ = run(inputs)
    return loss
